# revision 1
# baseline (speedup 1.0000x reference)
"""Trainium2 Bass kernel for the MTGNN top-k adjacency masking problem.

Row-sharded across 8 NeuronCores; the device does the O(N^2) work, the host
does O(N*k) exact selection.

  host prep:  n1/n2 node factors via XLA (bitwise-matches the reference),
              per-element threshold q = atanh(T0 - 0.01*noise)/3 in bf16.
              By monotonicity, tanh(3a) + 0.01*noise > T0  <=>  a > q, so the
              device never needs tanh, the noise add, or fp32 noise traffic.
  device:     a = [n1|n2] @ [n2|-n1]^T row block (bf16 matmul, fp32 PSUM) ->
              VectorE is_gt(a, q) straight from PSUM -> u8 candidate mask.
              T0 = 1.0092 sits >= 32x the device score drift below every
              row's 32nd-largest score (min 1.009844 on the fixed inputs).
  host trim:  re-score the ~200 candidates/row in fp32 with XLA tanh and 
              keep the exact top-32 by (score desc, index asc), matching
              jax.lax.top_k tie-breaking; any row with <32 candidates (i.e.
              any conceivable threshold failure) is recomputed exactly.
"""
import os
import sys

import numpy as np

for _p in ("/opt/trn_rl_repo", os.path.expanduser("~/.axon_site/_ro/trn_rl_repo")):
    if os.path.isdir(_p) and _p not in sys.path:
        sys.path.insert(0, _p)

from concourse import bacc, mybir, tile  # noqa: E402
from concourse.bass_utils import run_bass_kernel_spmd  # noqa: E402


def _register_ntff_hook():
    """Provide antenv.axon_hooks via a sys.modules shim so
    run_bass_kernel_spmd(trace=True) can capture NTFF profiles through the
    libaxon_pjrt.so C ABI (mirrors trn_agent_boot's ctypes hook)."""
    try:
        from antenv.axon_hooks import get_axon_ntff_profile_hook  # noqa: F401
        return  # real module present
    except ImportError:
        pass
    import contextlib
    import ctypes
    import types

    so_path = "/opt/axon/libaxon_pjrt.so"
    if not os.path.exists(so_path):
        return
    lib = ctypes.CDLL(so_path)
    if not hasattr(lib, "axon_start_nrt_profile"):
        return
    lib.axon_start_nrt_profile.argtypes = [
        ctypes.POINTER(ctypes.c_int64), ctypes.c_size_t]
    lib.axon_start_nrt_profile.restype = ctypes.c_int64
    lib.axon_stop_nrt_profile.argtypes = [ctypes.c_char_p]
    lib.axon_stop_nrt_profile.restype = ctypes.c_int64

    @contextlib.contextmanager
    def _hook(output_dir, device_ids):
        import jax
        jax.devices()
        if device_ids:
            ids = (ctypes.c_int64 * len(device_ids))(*device_ids)
            rc = lib.axon_start_nrt_profile(ids, len(device_ids))
        else:
            rc = lib.axon_start_nrt_profile(None, 0)
        if rc != 0:
            raise RuntimeError(f"axon_start_nrt_profile rc={rc}")
        try:
            yield
        finally:
            n = lib.axon_stop_nrt_profile(str(output_dir).encode())
            print(f"ntff profile: {n} file(s) -> {output_dir}", file=sys.stderr)

    mod = types.ModuleType("antenv.axon_hooks")
    mod.get_axon_ntff_profile_hook = lambda: _hook
    mod.set_axon_ntff_profile_hook = lambda h: None
    sys.modules["antenv.axon_hooks"] = mod


_register_ntff_hook()

N = 8192
DIM = 64
K = 32
ALPHA = np.float32(3.0)
M = 8                    # cores
ROWS = N // M            # rows per core (1024)
P = 128                  # partitions
NRT = ROWS // P          # row tiles per core (8)
SEG = 32                 # segment width for the seg-max prune
NSEG = N // SEG          # 256 segments per row
MMW = 512                # matmul moving free dim (one PSUM bank fp32)
PSW = 2048               # psum group width (4 banks)
CTW = P * NRT            # 1024 columns of lhsT block
MARGIN = np.float32(1e-4)  # threshold slack >> bf16-matmul score drift
# Fixed candidate threshold: min per-row 32nd-largest score is 1.009844 on the
# fixed-seed inputs; device score drift is <2e-5. Any row where this ever
# fails yields <32 candidates and is exactly recomputed on the host.
T0 = np.float32(1.0095)

f32 = mybir.dt.float32
bf16 = mybir.dt.bfloat16
u8 = mybir.dt.uint8

# per-row-tile engine for the score = adj + t add:
#   "dma"  -> SWDGE accum DMA (costs 2x on the DMA fabric, free for engines)
#   "dve"  -> plain chunk loads + VectorE tensor_tensor add
#   "pool" -> plain chunk loads + GpSimd tensor_tensor add
ADD_PLAN = ("dve", "pool", "dve", "pool", "dve", "pool", "dve", "pool")

_BUILT = None


# per-row-tile engine for the threshold compare (is_gt against q):
CMP_PLAN = ("dve",) * 8  # GpSimd cannot read PSUM


def _build():
    nc = bacc.Bacc(None, target_bir_lowering=False, debug=False)
    # single DRAM input for both matmul operands: [128, CTW | N]
    cd_in = nc.declare_dram_parameter("cd", [P, CTW + N], bf16, isOutput=False)
    qb_in = nc.declare_dram_parameter("qb", [ROWS, N], bf16, isOutput=False)
    out_d = nc.declare_dram_parameter("out", [ROWS, N], u8, isOutput=True)

    with tile.TileContext(nc) as tc:
        with (
            tc.tile_pool(name="const", bufs=1) as cpool,
            tc.tile_pool(name="outp", bufs=3) as opool,
            tc.tile_pool(name="qstage", bufs=10) as qpool,
            tc.tile_pool(name="psum", bufs=2, space="PSUM") as ppool,
        ):
            cd = cpool.tile([P, CTW + N], bf16)
            nc.sync.dma_start(out=cd[:], in_=cd_in[:])

            for rt in range(NRT):
                o = opool.tile([P, N], u8, tag="outp")
                cmp_e = nc.vector if CMP_PLAN[rt] == "dve" else nc.gpsimd
                for cc in range(N // PSW):
                    ps = ppool.tile([P, PSW], f32, tag="ps")
                    c0, c1 = PSW * cc, PSW * (cc + 1)
                    qt = qpool.tile([P, PSW], bf16, tag="qstage")
                    nc.sync.dma_start(
                        out=qt[:], in_=qb_in[P * rt : P * (rt + 1), c0:c1])
                    for q in range(PSW // MMW):
                        j0 = PSW * cc + MMW * q
                        nc.tensor.matmul(
                            ps[:, MMW * q : MMW * (q + 1)],
                            lhsT=cd[:, P * rt : P * (rt + 1)],
                            rhs=cd[:, CTW + j0 : CTW + j0 + MMW],
                            start=True, stop=True,
                        )
                    # candidate mask: 1 iff a > q  (<=> tanh(3a)+t > T0)
                    cmp_e.tensor_tensor(
                        o[:, c0:c1], ps[:], qt[:], mybir.AluOpType.is_gt)
                nc.sync.dma_start(out=out_d[P * rt : P * (rt + 1), :], in_=o[:])
    nc.compile()
    return nc


_JAX_FNS = None


def _jax_fns():
    """jax-on-neuron helpers so our numerics match the reference's XLA ops."""
    global _JAX_FNS
    if _JAX_FNS is None:
        import jax
        import jax.numpy as jnp

        dev = jax.devices()[0]

        @jax.jit
        def node_factor(e, w, b):
            return jnp.tanh(3.0 * (e @ w.T + b))

        @jax.jit
        def tanh3(a):
            return jnp.tanh(3.0 * a)

        _JAX_FNS = (jax, node_factor, tanh3, dev)
    return _JAX_FNS


def _host_prep(idx, emb1_w, emb2_w, w1, b1, w2, b2, noise):
    jax, node_factor, _, dev = _jax_fns()
    idx = np.asarray(idx)
    e1 = np.asarray(emb1_w, dtype=np.float32)[idx]
    e2 = np.asarray(emb2_w, dtype=np.float32)[idx]
    w1 = np.asarray(w1, dtype=np.float32)
    b1 = np.asarray(b1, dtype=np.float32)
    w2 = np.asarray(w2, dtype=np.float32)
    b2 = np.asarray(b2, dtype=np.float32)
    # bitwise-match the reference's n1/n2 (XLA on the same backend)
    n1 = np.asarray(jax.device_get(node_factor(e1, w1, b1)), dtype=np.float32)
    n2 = np.asarray(jax.device_get(node_factor(e2, w2, b2)), dtype=np.float32)
    C = np.concatenate([n1, n2], axis=1).astype(np.float32)        # [N, 128]
    D = np.concatenate([n2, -n1], axis=1).astype(np.float32).T     # [128, N]
    D = np.ascontiguousarray(D)
    t = (np.asarray(noise, dtype=np.float32) * np.float32(0.01)).astype(np.float32)
    return C, D, t


def _to_bf16_bits(x):
    """fp32 -> bf16 (round-to-nearest-even), returned as ml_dtypes/bf16 array."""
    import jax.numpy as jnp
    import jax
    with jax.default_device(jax.devices("cpu")[0]):
        return np.asarray(jnp.asarray(x).astype(jnp.bfloat16))


def _make_q(t):
    """Per-element matmul-space threshold: a > q  <=>  tanh(3a) + t > T0."""
    tt = (np.float32(T0) - t).astype(np.float32)
    q = np.full(tt.shape, 1e30, dtype=np.float32)
    m = tt < 1.0
    q[m] = (np.arctanh(tt[m]) / np.float32(3.0)).astype(np.float32)
    return q


def _run_device(C, D, q, trace=False):
    global _BUILT
    if _BUILT is None:
        _BUILT = _build()
    nc = _BUILT
    in_maps = []
    for c in range(M):
        r0 = c * ROWS
        cd = np.concatenate(
            [np.ascontiguousarray(C[r0 : r0 + ROWS].T), D], axis=1
        ).astype(np.float32)
        cdb = _to_bf16_bits(cd)
        in_maps.append({"cd": cdb, "qb": _to_bf16_bits(q[r0 : r0 + ROWS])})
    res = run_bass_kernel_spmd(nc, in_maps, list(range(M)), trace=trace)
    cand = np.empty((N, N), dtype=np.uint8)
    for c in range(M):
        r0 = c * ROWS
        # u8 cast of sign(): +1 -> 1, 0 -> 0, -1 -> 255; candidates are == 1
        cand[r0 : r0 + ROWS] = res.results[c]["out"] == 1
    return cand, res


def _host_trim(C, D, t, cand):
    """Exact per-row top-32 among device candidates, CPU-scored."""
    out = np.zeros((N, N), dtype=np.float32)
    rows, cols = np.nonzero(cand)
    # host re-score of candidates; tanh via XLA to match the reference impl
    jax, _, tanh3, dev = _jax_fns()
    Dt = np.ascontiguousarray(D.T)
    a = np.empty(len(rows), dtype=np.float32)
    B = 1 << 20
    for i in range(0, len(rows), B):
        a[i:i + B] = np.einsum(
            "nk,nk->n", C[rows[i:i + B]], Dt[cols[i:i + B]],
            dtype=np.float32, optimize=True)
    PAD = 1 << 20
    npad = ((len(a) + PAD - 1) // PAD) * PAD
    ap = np.zeros(npad, dtype=np.float32)
    ap[: len(a)] = a
    adjt = np.concatenate([
        np.asarray(jax.device_get(tanh3(ap[i : i + PAD])), dtype=np.float32)
        for i in range(0, npad, PAD)
    ])[: len(a)]
    adj = np.maximum(adjt, np.float32(0.0))
    sc = (adj + t[rows, cols]).astype(np.float32)

    counts = np.bincount(rows, minlength=N)
    bad_rows = set(np.nonzero(counts < K)[0].tolist())

    order = np.lexsort((cols, -sc.astype(np.float64), rows))
    rows_s, cols_s, adj_s = rows[order], cols[order], adj[order]
    starts = np.zeros(N + 1, dtype=np.int64)
    np.cumsum(counts, out=starts[1:])
    pos = np.arange(len(rows_s)) - starts[rows_s]
    sel = pos < K
    out[rows_s[sel], cols_s[sel]] = adj_s[sel]

    for r in bad_rows:  # exact fallback, exceedingly rare (verified empty)
        a_r = (C[r : r + 1] @ D).astype(np.float32).reshape(-1)
        adj_r = np.maximum(
            np.asarray(jax.device_get(tanh3(a_r)), dtype=np.float32), np.float32(0.0)
        )
        sc_r = (adj_r + t[r]).astype(np.float32)
        o = np.lexsort((np.arange(N), -sc_r.astype(np.float64)))[:K]
        out[r] = 0.0
        out[r, o] = adj_r[o]
    return out


def kernel(idx, emb1_w, emb2_w, w1, b1, w2, b2, noise):
    C, D, t = _host_prep(idx, emb1_w, emb2_w, w1, b1, w2, b2, noise)
    cand, _ = _run_device(C, D, _make_q(t), trace=False)
    return _host_trim(C, D, t, cand)


def kernel_profiled(idx, emb1_w, emb2_w, w1, b1, w2, b2, noise):
    """Same as kernel() but returns (out, BassKernelResults-with-profile)."""
    C, D, t = _host_prep(idx, emb1_w, emb2_w, w1, b1, w2, b2, noise)
    cand, res = _run_device(C, D, _make_q(t), trace=True)
    return _host_trim(C, D, t, cand), res

